# revision 32
# baseline (speedup 1.0000x reference)
"""Causal self-attention (GQA + RoPE) for TRN2, sharded over 8 NeuronCores.

Sharding: tensor-parallel over heads. Each core owns 4 query heads and 1 KV
head (H=32, HKV=8 -> group size 4). Column-parallel q/k/v projections,
row-parallel o_proj; the final all-reduce over the 8 partial [T, D] outputs
happens on the host after the gather.

v2 design notes (vs the f32r baseline):
  - All matmul operands are bf16: FWL weight loads, half the SBUF/DMA
    bandwidth, and much lower PE power (the f32r version spent ~60% of its
    runtime HAM-throttled to 1.2 GHz).
  - Scores for an even/odd head pair run as two concurrent 64-row
    tile_position matmuls (kT is duplicated on both partition halves).
  - attn@v is computed transposed: stationary = ex [keys, t-block], moving =
    v-augmented [keys, 64 v dims + ones col] -> yt [t, 65] with the softmax
    denominator landing in column 64.  Normalization is then a per-partition
    reciprocal_approx_fast + tensor_scalar_mul on the DVE (no ACT table
    thrash, no PE broadcast, no cross-partition shifts).
  - RoPE pair rotation and the kT partition-duplication are PE permutation
    matmuls (q_rot = I@qc + P@qs accumulated in PSUM), not SBUF->SBUF DMAs.
  - Inputs are loaded with 4-chunk batched DMA descriptors, all issued
    up-front on the sync engine; output partials are staged to bf16 and
    stored 2048-cols-at-a-time from the gpsimd engine.
"""

import math

import numpy as np
import ml_dtypes

import concourse.bass as bass
import concourse.mybir as mybir
import concourse.tile as tile
from concourse import bacc

D = 2048
H = 32
HKV = 8
HD = 64
T = 2048
NCORES = 8
HPC = H // NCORES        # 4 query heads per core
QC = HPC * HD            # 256 q dims per core
ROPE_BASE = 10000.0
S = 512                  # t-strip width
NSTRIP = T // S          # 4
KC = D // 128            # 16 contraction chunks
KG = 4                   # kc chunks per batched DMA / SBUF tile

F32 = mybir.dt.float32
BF16 = mybir.dt.bfloat16

BF = ml_dtypes.bfloat16


def _build_kernel(debug=False):
    nc = bacc.Bacc("TRN2", target_bir_lowering=False, debug=False,
                   num_devices=NCORES)

    # xP/wqP/wkvP are host-packed to the exact SBUF tile layout so each DMA
    # descriptor row is one contiguous 2-4KB segment (the DMA engines are
    # descriptor-rate limited at ~74ns/row-segment, so segment size sets
    # the effective input bandwidth)
    xP = nc.dram_tensor("xP", [NSTRIP * 4 * 128, KG * S], BF16,
                        kind="ExternalInput").ap()
    wqP = nc.dram_tensor("wqP", [4 * 128, KG * QC], BF16,
                         kind="ExternalInput").ap()
    wkvP = nc.dram_tensor("wkvP", [4 * 128, KG * 128], BF16,
                          kind="ExternalInput").ap()
    woT = nc.dram_tensor("woT", [QC, D], BF16, kind="ExternalInput").ap()
    # compact rope tables: [32, T]; duplicated to 128 partitions on-chip
    # (the rotation sign pattern is baked into the permutation matrices)
    cosT = nc.dram_tensor("cosT", [32, T], F32, kind="ExternalInput").ap()
    sinT = nc.dram_tensor("sinT", [32, T], F32, kind="ExternalInput").ap()
    pqM = nc.dram_tensor("pqM", [128, 128], BF16, kind="ExternalInput").ap()
    pkM = nc.dram_tensor("pkM", [64, 2, 128], BF16, kind="ExternalInput").ap()
    idM = nc.dram_tensor("idM", [128, 128], BF16, kind="ExternalInput").ap()
    out = nc.dram_tensor("out", [T, D], BF16, kind="ExternalOutput").ap()
    dbg = {}
    if debug:
        for nm, shp, dt in [("d_qT0", [128, T], BF16), ("d_qT1", [128, T], BF16),
                            ("d_kT", [128, T], BF16),
                            ("d_vaug", [128, 16 * 72], BF16),
                            ("d_ex", [128, 4 * S], BF16),
                            ("d_yt", [128, 8 * 128], F32),
                            ("d_ytn0", [128, S], BF16),
                            ("d_ytn1", [128, S], BF16)]:
            dbg[nm] = nc.dram_tensor(nm, shp, dt, kind="ExternalOutput").ap()

    with tile.TileContext(nc) as tc:
        with (
            tc.tile_pool(name="consts", bufs=1) as consts,
            tc.tile_pool(name="persist", bufs=1) as persist,
            tc.tile_pool(name="rtmp", bufs=6) as rtmp,
            tc.tile_pool(name="expp", bufs=6) as expp,
            tc.tile_pool(name="ypk", bufs=3) as ypkp,
            tc.tile_pool(name="rcp", bufs=6) as rcpp,
            tc.tile_pool(name="ytn", bufs=4) as ytnp,
            tc.tile_pool(name="stg", bufs=3) as stgp,
            tc.tile_pool(name="mm", bufs=3, space="PSUM") as mmp,
            tc.tile_pool(name="ytp", bufs=2, space="PSUM") as ytp,
            tc.tile_pool(name="tpp", bufs=1, space="PSUM") as tpp,
        ):
            # ---- input DMAs: all issued up-front on the sync engine, in
            # criticality order (strip-0 weights+x first, wo last) ----
            ident = consts.tile([128, 128], BF16)
            nc.sync.dma_start(out=ident, in_=idM)
            pq_sb = consts.tile([128, 128], BF16)
            pk_sb = consts.tile([64, 2, 128], BF16)
            nc.sync.dma_start(out=pq_sb, in_=pqM)
            nc.sync.dma_start(out=pk_sb, in_=pkM)
            # rope tables first: everything downstream of rope blocks on them
            # (4 parallel block loads per table, no serial dup chain)
            cs_c = consts.tile([128, T], F32)
            cs_s = consts.tile([128, T], F32)
            for b in range(4):
                nc.sync.dma_start(out=cs_c[b * 32:(b + 1) * 32, :], in_=cosT)
                nc.sync.dma_start(out=cs_s[b * 32:(b + 1) * 32, :], in_=sinT)
            # per-group tiles (4 chunks each) with 2-4KB contiguous rows:
            # fine-grained enough that projections start as groups land,
            # coarse enough to stay byte-limited on the DMA pool
            wq_sb = [consts.tile([128, KG, QC], BF16, name=f"wq{g}")
                     for g in range(KC // KG)]
            wkv_sb = [consts.tile([128, KG, 128], BF16, name=f"wkv{g}")
                      for g in range(KC // KG)]
            xa = {}  # (strip, g) -> [128, KG, S] tile

            def load_xa(strip):
                for g in range(KC // KG):
                    xt = consts.tile([128, KG, S], BF16, name=f"xa{strip}_{g}")
                    r0 = (strip * 4 + g) * 128
                    nc.sync.dma_start(
                        out=xt.rearrange("p c t -> p (c t)"),
                        in_=xP[r0:r0 + 128, :])
                    xa[strip, g] = xt

            for g in range(KC // KG):
                nc.sync.dma_start(
                    out=wq_sb[g].rearrange("p c n -> p (c n)"),
                    in_=wqP[g * 128:(g + 1) * 128, :])
                nc.sync.dma_start(
                    out=wkv_sb[g].rearrange("p c n -> p (c n)"),
                    in_=wkvP[g * 128:(g + 1) * 128, :])
                xt = consts.tile([128, KG, S], BF16, name=f"xa0_{g}")
                nc.sync.dma_start(
                    out=xt.rearrange("p c t -> p (c t)"),
                    in_=xP[g * 128:(g + 1) * 128, :])
                xa[0, g] = xt
            load_xa(1)
            load_xa(2)
            wo_sb = consts.tile([128, 2, D], BF16)
            nc.sync.dma_start(
                out=wo_sb, in_=woT.rearrange("(c p) n -> p c n", p=128))
            load_xa(3)

            # PE warmup: lift the HAM cold throttle while input DMAs land
            junk = consts.tile([128, 512], BF16)
            nc.vector.memset(junk, 1.0)
            # zeros stationary: used to clear whole PSUM banks in one matmul
            # (start=True marks the full 2KB bank pending-zero, so packing
            # several accumulation slots per bank requires a single
            # bank-covering start op; the per-slot matmuls then accumulate
            # with start=False)
            zeros_b = consts.tile([128, 128], BF16)
            nc.vector.memset(zeros_b, 0.0)
            # enough warmup to both lift the HAM throttle and keep the PE
            # busy while the first ~7MB of inputs stream in (~25us at the
            # observed ~270GB/s aggregate DMA rate)
            warm_ps = mmp.tile([128, 512], F32, tag="mm", name="warm")
            for w in range(44):
                nc.tensor.matmul(
                    warm_ps, junk[:, 0:128], junk,
                    start=True, stop=True, skip_group_check=True)

            # persistent activations
            qT = [persist.tile([128, T], BF16, name=f"qT{i}") for i in range(2)]
            # k duplicated on both partition halves so the head-pair scores
            # run as two concurrent 64-row tile_position matmuls
            kT = persist.tile([128, T], BF16)
            vaug = persist.tile([128, 4 * NSTRIP, 72], BF16)
            ones_f = consts.tile([128, 4 * NSTRIP, 1], F32)
            nc.vector.memset(ones_f, 1.0)
            nc.vector.tensor_copy(vaug[:, :, 64:65], ones_f)

            def proj_filler(strip):
                """Yield closures, one PE op each, for this strip's q/kv
                projection; rope + v-transpose work rides along."""
                t0 = strip * S
                tsl = slice(t0, t0 + S)

                def rope_q(hp, pq):
                    qc = rtmp.tile([128, S], F32, tag="rtf", bufs=3,
                                   name=f"qc{strip}{hp}")
                    qs = rtmp.tile([128, S], BF16, tag="rt", name=f"qs{strip}{hp}")
                    nc.vector.tensor_mul(qc, pq, cs_c[:, tsl])
                    nc.vector.tensor_mul(qs, pq, cs_s[:, tsl])
                    rps = mmp.tile([128, S], F32, tag="mm", name=f"rq{strip}{hp}")
                    nc.tensor.matmul(rps, pq_sb, qs, start=True, stop=True,
                                     skip_group_check=True)
                    nc.vector.tensor_add(qT[hp][:, tsl], qc, rps)

                for hp in range(2):
                    pq = mmp.tile([128, S], F32, tag="mm", name=f"pq{strip}_{hp}")
                    for kc in range(KC):
                        def mk(hp=hp, pq=pq, kc=kc):
                            nc.tensor.matmul(
                                pq,
                                wq_sb[kc // KG][:, kc % KG,
                                                hp * 128:(hp + 1) * 128],
                                xa[strip, kc // KG][:, kc % KG, :],
                                start=(kc == 0), stop=(kc == KC - 1))
                            if kc == KC - 1:
                                rope_q(hp, pq)
                        yield mk

                pkv = mmp.tile([128, S], F32, tag="mm", name=f"pkv{strip}")
                state = {}

                def rope_kv():
                    kc_t = rtmp.tile([128, S], BF16, tag="rt", name=f"kc{strip}")
                    ks_t = rtmp.tile([128, S], BF16, tag="rt", name=f"ks{strip}")
                    nc.vector.tensor_mul(
                        kc_t[0:64, :], pkv[0:64, :], cs_c[0:64, tsl])
                    nc.vector.tensor_mul(
                        ks_t[0:64, :], pkv[0:64, :], cs_s[0:64, tsl])
                    kps = mmp.tile([128, S], F32, tag="mm", name=f"kd{strip}")
                    nc.tensor.matmul(kps, pk_sb[:, 0, :], kc_t[0:64, :],
                                     start=True, stop=False,
                                     skip_group_check=True)
                    nc.tensor.matmul(kps, pk_sb[:, 1, :], ks_t[0:64, :],
                                     start=False, stop=True,
                                     skip_group_check=True)
                    nc.vector.tensor_copy(kT[:, tsl], kps)
                    vt_s = rtmp.tile([128, S], BF16, tag="vt", name=f"vt{strip}")
                    nc.vector.tensor_copy(vt_s[64:128, :], pkv[64:128, :])
                    state["vt"] = vt_s

                for kc in range(KC):
                    def mk(kc=kc):
                        nc.tensor.matmul(
                            pkv, wkv_sb[kc // KG][:, kc % KG, :],
                            xa[strip, kc // KG][:, kc % KG, :],
                            start=(kc == 0), stop=(kc == KC - 1))
                        if kc == KC - 1:
                            rope_kv()
                    yield mk
                for n in range(4):
                    def mk(n=n):
                        if n == 0:
                            state["pt"] = tpp.tile([128, 4, 128], BF16,
                                                   tag="tp", name=f"pt{strip}")
                        pt = state["pt"]
                        nc.tensor.transpose(
                            pt[:, n, 0:64],
                            state["vt"][64:128, n * 128:(n + 1) * 128],
                            ident[64:128, 64:128])
                        nc.vector.tensor_copy(
                            vaug[:, strip * 4 + n, 0:64], pt[:, n, 0:64])
                    yield mk

            def oproj_filler(strip, ytn):
                """Yield closures, one o_proj matmul each; eviction + store
                ride along after each accumulation group."""
                t0 = strip * S
                for tsub in range(4):
                    trow = t0 + tsub * 128
                    stage = stgp.tile([128, 4, S], BF16, tag="st",
                                      name=f"st{strip}{tsub}")
                    for n in range(4):
                        po = mmp.tile([128, S], F32, tag="mm",
                                      name=f"po{strip}{tsub}{n}")
                        for c in range(2):
                            def mk(po=po, c=c, tsub=tsub, n=n, trow=trow,
                                   stage=stage):
                                nc.tensor.matmul(
                                    po,
                                    ytn[c][:, tsub * 128:(tsub + 1) * 128],
                                    wo_sb[:, c, n * S:(n + 1) * S],
                                    start=(c == 0), stop=(c == 1),
                                    skip_group_check=True)
                                if c == 1:
                                    # evictions on DVE: the ACT engine is the
                                    # binding engine during late-strip
                                    # attention (2 EXPs per j > PE per-j work)
                                    nc.vector.tensor_copy(
                                        stage[:, n, :], po)
                                    if n == 3:
                                        nc.gpsimd.dma_start(
                                            out=out[trow:trow + 128, :],
                                            in_=stage)
                            yield mk

            def run_filler(filler, frac):
                import itertools
                for fn in itertools.islice(filler, frac):
                    fn()

            # strip 0 projection runs dense (nothing to overlap with)
            for fn in proj_filler(0):
                fn()

            ytn_strips = {}
            fillers = []

            for strip in range(NSTRIP):
                t0 = strip * S
                n_sc = (strip + 1) * 4
                ytn = [ytnp.tile([128, S], BF16, tag="ytn",
                                 name=f"ytn{strip}{c}") for c in range(2)]
                ytn_strips[strip] = ytn

                pending = 0
                if strip + 1 < NSTRIP:
                    fillers.append(proj_filler(strip + 1))
                    pending += 52
                if strip - 1 >= 0:
                    fillers.append(oproj_filler(strip - 1, ytn_strips[strip - 1]))
                    pending += 32
                n_chunks = 2 * n_sc

                import itertools
                filler_iter = itertools.chain(*fillers)
                fillers = [filler_iter]
                per_chunk = -(-pending // n_chunks) if pending else 0

                for hp in range(2):
                    yt = ytp.tile([128, 8, 128], F32, tag="yt",
                                  name=f"yt{strip}{hp}")
                    # zero both banks of yt with one full-bank matmul each
                    for half in range(2):
                        nc.tensor.matmul(
                            yt[:, half * 4:(half + 1) * 4, :],
                            zeros_b, junk,
                            start=True, stop=True, skip_group_check=True)
                    def emit_av(j, o, ex):
                        for h in range(2):
                            for tb in range(o // 128, 4):
                                nc.tensor.matmul(
                                    yt[:, h * 4 + tb, 0:65],
                                    ex[h][:, tb * 128:(tb + 1) * 128],
                                    vaug[:, j, 0:65],
                                    start=False,
                                    stop=(j == 4 * strip + tb),
                                    skip_group_check=True)

                    prev = None
                    for j in range(n_sc):
                        o = max(j * 128 - t0, 0)
                        jsl = slice(j * 128, (j + 1) * 128)
                        ex = [expp.tile([128, S], BF16, tag="exp",
                                        name=f"e{strip}{hp}{j}{h}")
                              for h in range(2)]
                        sc = [mmp.tile([128, S], F32, tag="mm",
                                       name=f"s{strip}{hp}{j}{h}")
                              for h in range(2)]
                        # both head matmuls adjacent: they occupy disjoint
                        # 64-row groups of the PE and run concurrently
                        for h in range(2):
                            lo = h * 64
                            nc.tensor.matmul(
                                sc[h][:, o:S],
                                kT[lo:lo + 64, jsl],
                                qT[hp][lo:lo + 64, t0 + o:t0 + S],
                                start=True, stop=True)
                        for h in range(2):
                            nc.scalar.activation(
                                ex[h][:, o:S], sc[h][:, o:S],
                                mybir.ActivationFunctionType.Exp,
                                scale=1.0 / math.sqrt(HD))
                            if j * 128 - t0 >= 0:
                                nc.gpsimd.affine_select(
                                    out=ex[h][:, o:o + 128],
                                    in_=ex[h][:, o:o + 128],
                                    pattern=[[1, 128]], base=0,
                                    channel_multiplier=-1,
                                    compare_op=mybir.AluOpType.is_ge,
                                    fill=0.0)
                        # attn@v for the previous j: its exps are ready, so
                        # the PE never waits on the ACT engine
                        if prev is not None:
                            emit_av(*prev)
                        prev = (j, o, ex)
                        if debug and strip == 0 and hp == 0:
                            nc.sync.dma_start(
                                out=dbg["d_ex"][:, j * S:(j + 1) * S],
                                in_=ex[0])
                        run_filler(filler_iter, per_chunk)
                    emit_av(*prev)

                    # normalize by the denominator in column 64, pack the
                    # head pair side by side, transpose back to [d, t]
                    if debug and strip == 0 and hp == 0:
                        ydbg = rtmp.tile([128, 8 * 128], F32, tag="yd",
                                         bufs=1, name="ydbg")
                        nc.vector.tensor_copy(
                            ydbg, yt.rearrange("p a b -> p (a b)"))
                        nc.sync.dma_start(out=dbg["d_yt"], in_=ydbg)
                    tp = tpp.tile([128, 4, 128], BF16, tag="tp",
                                  name=f"tp{strip}{hp}")
                    for tb in range(4):
                        ypk = ypkp.tile([128, 128], BF16, tag="yp",
                                        name=f"yp{strip}{hp}{tb}")
                        for h in range(2):
                            rcp = rcpp.tile([128, 1], F32, tag="rc",
                                            name=f"rc{strip}{hp}{tb}{h}")
                            nc.vector.reciprocal_approx_fast(
                                rcp, yt[:, h * 4 + tb, 64:65])
                            nc.vector.tensor_scalar_mul(
                                ypk[:, h * 64:(h + 1) * 64],
                                yt[:, h * 4 + tb, 0:64], rcp)
                        nc.tensor.transpose(tp[:, tb, :], ypk, ident)
                        nc.vector.tensor_copy(
                            ytn[hp][:, tb * 128:(tb + 1) * 128],
                            tp[:, tb, :])

                if debug and strip == 0:
                    nc.sync.dma_start(out=dbg["d_qT0"], in_=qT[0])
                    nc.sync.dma_start(out=dbg["d_qT1"], in_=qT[1])
                    nc.sync.dma_start(out=dbg["d_kT"], in_=kT)
                    nc.sync.dma_start(
                        out=dbg["d_vaug"],
                        in_=vaug.rearrange("p a b -> p (a b)"))
                    nc.sync.dma_start(out=dbg["d_ytn0"], in_=ytn[0])
                    nc.sync.dma_start(out=dbg["d_ytn1"], in_=ytn[1])

                for fn in filler_iter:
                    fn()
                fillers = []

            # last strip's o_proj runs dense at the tail
            for fn in oproj_filler(NSTRIP - 1, ytn_strips[NSTRIP - 1]):
                fn()

    nc.compile()
    return nc


_NC_CACHE = {}


def _get_nc(debug=False):
    if debug not in _NC_CACHE:
        _NC_CACHE[debug] = _build_kernel(debug=debug)
    return _NC_CACHE[debug]


def _host_consts():
    # signed rope permutation matrices: the pair rotation is
    # rot = cos-part + P @ sin-part, with the [-sin | +sin] sign pattern of
    # the x1/x2 halves baked into P so the sin table stays all-positive
    def swap64(m):
        return (m & ~63) | ((m + 32) & 63)

    def sgn(m):
        return -1.0 if (m % 64) < 32 else 1.0

    pq = np.zeros((128, 128), dtype=np.float32)
    for m in range(128):
        pq[swap64(m), m] = sgn(m)
    pk = np.zeros((64, 2, 128), dtype=np.float32)
    for m in range(128):
        pk[m % 64, 0, m] = 1.0
        pk[(m % 64 + 32) % 64, 1, m] = sgn(m)
    ident = np.eye(128, dtype=np.float32)

    theta = 1.0 / ROPE_BASE ** (np.arange(0, HD, 2, dtype=np.float64) / HD)
    ang = np.arange(T, dtype=np.float64)[None, :] * theta[:, None]  # [32, T]
    cosT = np.ascontiguousarray(np.cos(ang).astype(np.float32))
    sinT = np.ascontiguousarray(np.sin(ang).astype(np.float32))
    return (pq.astype(BF), pk.astype(BF), ident.astype(BF), cosT, sinT)


def _prep_inputs(x, wq, wk, wv, wo):
    """Host-side shard + layout prep. Returns per-core input maps."""
    x = np.asarray(x, dtype=np.float32).reshape(T, D)
    wq = np.asarray(wq, dtype=np.float32)
    wk = np.asarray(wk, dtype=np.float32)
    wv = np.asarray(wv, dtype=np.float32)
    wo = np.asarray(wo, dtype=np.float32)

    # pack x to the SBUF tile layout: row s*128+p, col kc*S+t holds
    # xT[kc*128+p, s*S+t]  (see kernel load_xa; 16KB contiguous rows)
    xp = x.T.reshape(4, KG, 128, NSTRIP, S).transpose(3, 0, 2, 1, 4)
    xP = np.ascontiguousarray(
        xp.reshape(NSTRIP * 4 * 128, KG * S)).astype(BF)

    def pack_w(wT, n):  # wT: [D, n] -> [4*128, KG*n]
        wp = wT.reshape(4, KG, 128, n).transpose(0, 2, 1, 3)
        return np.ascontiguousarray(wp.reshape(4 * 128, KG * n)).astype(BF)

    # head-dim permutation for rope: [even pair comps | odd pair comps]
    perm = np.concatenate([np.arange(0, HD, 2), np.arange(1, HD, 2)])
    pqM, pkM, idM, cosT, sinT = _host_consts()

    in_maps = []
    for c in range(NCORES):
        wq_c = wq[c * QC:(c + 1) * QC].reshape(HPC, HD, D)[:, perm, :]
        wq_c = wq_c.reshape(QC, D)
        wk_c = wk[c * HD:(c + 1) * HD][perm, :]
        wv_c = wv[c * HD:(c + 1) * HD]
        wkv_c = np.concatenate([wk_c, wv_c], axis=0)          # [128, D]
        wo_c = wo[:, c * QC:(c + 1) * QC]                      # [D, QC]
        in_maps.append({
            "xP": xP,
            "wqP": pack_w(np.ascontiguousarray(wq_c.T), QC),
            "wkvP": pack_w(np.ascontiguousarray(wkv_c.T), 128),
            "woT": np.ascontiguousarray(wo_c.T).astype(BF),
            "cosT": cosT,
            "sinT": sinT,
            "pqM": pqM,
            "pkM": pkM,
            "idM": idM,
        })
    return in_maps


def kernel(x, wq, wk, wv, wo):
    from concourse.bass_utils import run_bass_kernel_spmd

    nc = _get_nc()
    in_maps = _prep_inputs(x, wq, wk, wv, wo)
    res = run_bass_kernel_spmd(nc, in_maps, core_ids=list(range(NCORES)))
    acc = np.zeros((T, D), dtype=np.float64)
    for c in range(NCORES):
        acc += res.results[c]["out"].astype(np.float64)
    return acc.astype(np.float32).reshape(1, T, D)


# revision 34
# speedup vs baseline: 1.0067x; 1.0067x over previous
"""Causal self-attention (GQA + RoPE) for TRN2, sharded over 8 NeuronCores.

Sharding: tensor-parallel over heads. Each core owns 4 query heads and 1 KV
head (H=32, HKV=8 -> group size 4). Column-parallel q/k/v projections,
row-parallel o_proj; the final all-reduce over the 8 partial [T, D] outputs
happens on the host after the gather.

v2 design notes (vs the f32r baseline):
  - All matmul operands are bf16: FWL weight loads, half the SBUF/DMA
    bandwidth, and much lower PE power (the f32r version spent ~60% of its
    runtime HAM-throttled to 1.2 GHz).
  - Scores for an even/odd head pair run as two concurrent 64-row
    tile_position matmuls (kT is duplicated on both partition halves).
  - attn@v is computed transposed: stationary = ex [keys, t-block], moving =
    v-augmented [keys, 64 v dims + ones col] -> yt [t, 65] with the softmax
    denominator landing in column 64.  Normalization is then a per-partition
    reciprocal_approx_fast + tensor_scalar_mul on the DVE (no ACT table
    thrash, no PE broadcast, no cross-partition shifts).
  - RoPE pair rotation and the kT partition-duplication are PE permutation
    matmuls (q_rot = I@qc + P@qs accumulated in PSUM), not SBUF->SBUF DMAs.
  - Inputs are loaded with 4-chunk batched DMA descriptors, all issued
    up-front on the sync engine; output partials are staged to bf16 and
    stored 2048-cols-at-a-time from the gpsimd engine.
"""

import math

import numpy as np
import ml_dtypes

import concourse.bass as bass
import concourse.mybir as mybir
import concourse.tile as tile
from concourse import bacc

D = 2048
H = 32
HKV = 8
HD = 64
T = 2048
NCORES = 8
HPC = H // NCORES        # 4 query heads per core
QC = HPC * HD            # 256 q dims per core
ROPE_BASE = 10000.0
S = 512                  # t-strip width
NSTRIP = T // S          # 4
KC = D // 128            # 16 contraction chunks
KG = 4                   # kc chunks per batched DMA / SBUF tile

F32 = mybir.dt.float32
BF16 = mybir.dt.bfloat16

BF = ml_dtypes.bfloat16


def _build_kernel(debug=False):
    nc = bacc.Bacc("TRN2", target_bir_lowering=False, debug=False,
                   num_devices=NCORES)

    # xP/wqP/wkvP are host-packed to the exact SBUF tile layout so each DMA
    # descriptor row is one contiguous 2-4KB segment (the DMA engines are
    # descriptor-rate limited at ~74ns/row-segment, so segment size sets
    # the effective input bandwidth)
    xP = nc.dram_tensor("xP", [NSTRIP * 4 * 128, KG * S], BF16,
                        kind="ExternalInput").ap()
    wqP = nc.dram_tensor("wqP", [4 * 128, KG * QC], BF16,
                         kind="ExternalInput").ap()
    wkvP = nc.dram_tensor("wkvP", [4 * 128, KG * 128], BF16,
                          kind="ExternalInput").ap()
    woT = nc.dram_tensor("woT", [QC, D], BF16, kind="ExternalInput").ap()
    # compact rope tables: [32, T]; duplicated to 128 partitions on-chip
    # (the rotation sign pattern is baked into the permutation matrices)
    cosT = nc.dram_tensor("cosT", [32, T], F32, kind="ExternalInput").ap()
    sinT = nc.dram_tensor("sinT", [32, T], F32, kind="ExternalInput").ap()
    pqM = nc.dram_tensor("pqM", [128, 128], BF16, kind="ExternalInput").ap()
    pkM = nc.dram_tensor("pkM", [64, 2, 128], BF16, kind="ExternalInput").ap()
    idM = nc.dram_tensor("idM", [128, 128], BF16, kind="ExternalInput").ap()
    out = nc.dram_tensor("out", [T, D], BF16, kind="ExternalOutput").ap()
    dbg = {}
    if debug:
        for nm, shp, dt in [("d_qT0", [128, T], BF16), ("d_qT1", [128, T], BF16),
                            ("d_kT", [128, T], BF16),
                            ("d_vaug", [128, 16 * 72], BF16),
                            ("d_ex", [128, 4 * S], BF16),
                            ("d_yt", [128, 8 * 128], F32),
                            ("d_ytn0", [128, S], BF16),
                            ("d_ytn1", [128, S], BF16)]:
            dbg[nm] = nc.dram_tensor(nm, shp, dt, kind="ExternalOutput").ap()

    with tile.TileContext(nc) as tc:
        with (
            tc.tile_pool(name="consts", bufs=1) as consts,
            tc.tile_pool(name="persist", bufs=1) as persist,
            tc.tile_pool(name="rtmp", bufs=6) as rtmp,
            tc.tile_pool(name="expp", bufs=6) as expp,
            tc.tile_pool(name="ypk", bufs=3) as ypkp,
            tc.tile_pool(name="rcp", bufs=6) as rcpp,
            tc.tile_pool(name="ytn", bufs=4) as ytnp,
            tc.tile_pool(name="stg", bufs=3) as stgp,
            tc.tile_pool(name="mm", bufs=3, space="PSUM") as mmp,
            tc.tile_pool(name="ytp", bufs=2, space="PSUM") as ytp,
            tc.tile_pool(name="tpp", bufs=1, space="PSUM") as tpp,
        ):
            # ---- input DMAs: all issued up-front on the sync engine, in
            # criticality order (strip-0 weights+x first, wo last) ----
            ident = consts.tile([128, 128], BF16)
            nc.sync.dma_start(out=ident, in_=idM)
            pq_sb = consts.tile([128, 128], BF16)
            pk_sb = consts.tile([64, 2, 128], BF16)
            nc.sync.dma_start(out=pq_sb, in_=pqM)
            nc.sync.dma_start(out=pk_sb, in_=pkM)
            # rope tables first: everything downstream of rope blocks on them
            # (4 parallel block loads per table, no serial dup chain)
            cs_c = consts.tile([128, T], F32)
            cs_s = consts.tile([128, T], F32)
            for b in range(4):
                nc.sync.dma_start(out=cs_c[b * 32:(b + 1) * 32, :], in_=cosT)
                nc.sync.dma_start(out=cs_s[b * 32:(b + 1) * 32, :], in_=sinT)
            # per-group tiles (4 chunks each) with 2-4KB contiguous rows:
            # fine-grained enough that projections start as groups land,
            # coarse enough to stay byte-limited on the DMA pool
            wq_sb = [consts.tile([128, KG, QC], BF16, name=f"wq{g}")
                     for g in range(KC // KG)]
            wkv_sb = [consts.tile([128, KG, 128], BF16, name=f"wkv{g}")
                      for g in range(KC // KG)]
            xa = {}  # (strip, g) -> [128, KG, S] tile

            def load_xa(strip):
                for g in range(KC // KG):
                    xt = consts.tile([128, KG, S], BF16, name=f"xa{strip}_{g}")
                    r0 = (strip * 4 + g) * 128
                    nc.sync.dma_start(
                        out=xt.rearrange("p c t -> p (c t)"),
                        in_=xP[r0:r0 + 128, :])
                    xa[strip, g] = xt

            for g in range(KC // KG):
                nc.sync.dma_start(
                    out=wq_sb[g].rearrange("p c n -> p (c n)"),
                    in_=wqP[g * 128:(g + 1) * 128, :])
                nc.sync.dma_start(
                    out=wkv_sb[g].rearrange("p c n -> p (c n)"),
                    in_=wkvP[g * 128:(g + 1) * 128, :])
                xt = consts.tile([128, KG, S], BF16, name=f"xa0_{g}")
                nc.sync.dma_start(
                    out=xt.rearrange("p c t -> p (c t)"),
                    in_=xP[g * 128:(g + 1) * 128, :])
                xa[0, g] = xt
            load_xa(1)
            load_xa(2)
            wo_sb = consts.tile([128, 2, D], BF16)
            nc.sync.dma_start(
                out=wo_sb, in_=woT.rearrange("(c p) n -> p c n", p=128))
            load_xa(3)

            # PE warmup: lift the HAM cold throttle while input DMAs land
            junk = consts.tile([128, 512], BF16)
            nc.vector.memset(junk, 1.0)
            # zeros stationary: used to clear whole PSUM banks in one matmul
            # (start=True marks the full 2KB bank pending-zero, so packing
            # several accumulation slots per bank requires a single
            # bank-covering start op; the per-slot matmuls then accumulate
            # with start=False)
            zeros_b = consts.tile([128, 128], BF16)
            nc.vector.memset(zeros_b, 0.0)
            # enough warmup to both lift the HAM throttle and keep the PE
            # busy while the first ~7MB of inputs stream in (~25us at the
            # observed ~270GB/s aggregate DMA rate)
            warm_ps = mmp.tile([128, 512], F32, tag="mm", name="warm")
            for w in range(44):
                nc.tensor.matmul(
                    warm_ps, junk[:, 0:128], junk,
                    start=True, stop=True, skip_group_check=True)

            # persistent activations
            qT = [persist.tile([128, T], BF16, name=f"qT{i}") for i in range(2)]
            # k duplicated on both partition halves so the head-pair scores
            # run as two concurrent 64-row tile_position matmuls
            kT = persist.tile([128, T], BF16)
            vaug = persist.tile([128, 4 * NSTRIP, 72], BF16)
            ones_f = consts.tile([128, 4 * NSTRIP, 1], F32)
            nc.vector.memset(ones_f, 1.0)
            nc.vector.tensor_copy(vaug[:, :, 64:65], ones_f)

            def proj_filler(strip):
                """Yield closures, one PE op each, for this strip's q/kv
                projection; rope + v-transpose work rides along."""
                t0 = strip * S
                tsl = slice(t0, t0 + S)

                def rope_q(hp, pq):
                    qc = rtmp.tile([128, S], F32, tag="rtf", bufs=3,
                                   name=f"qc{strip}{hp}")
                    qs = rtmp.tile([128, S], BF16, tag="rt", name=f"qs{strip}{hp}")
                    nc.vector.tensor_mul(qc, pq, cs_c[:, tsl])
                    nc.vector.tensor_mul(qs, pq, cs_s[:, tsl])
                    rps = mmp.tile([128, S], F32, tag="mm", name=f"rq{strip}{hp}")
                    nc.tensor.matmul(rps, pq_sb, qs, start=True, stop=True,
                                     skip_group_check=True)
                    nc.vector.tensor_add(qT[hp][:, tsl], qc, rps)

                for hp in range(2):
                    pq = mmp.tile([128, S], F32, tag="mm", name=f"pq{strip}_{hp}")
                    for kc in range(KC):
                        def mk(hp=hp, pq=pq, kc=kc):
                            nc.tensor.matmul(
                                pq,
                                wq_sb[kc // KG][:, kc % KG,
                                                hp * 128:(hp + 1) * 128],
                                xa[strip, kc // KG][:, kc % KG, :],
                                start=(kc == 0), stop=(kc == KC - 1))
                            if kc == KC - 1:
                                rope_q(hp, pq)
                        yield mk

                pkv = mmp.tile([128, S], F32, tag="mm", name=f"pkv{strip}")
                state = {}

                def rope_kv():
                    kc_t = rtmp.tile([128, S], BF16, tag="rt", name=f"kc{strip}")
                    ks_t = rtmp.tile([128, S], BF16, tag="rt", name=f"ks{strip}")
                    nc.vector.tensor_mul(
                        kc_t[0:64, :], pkv[0:64, :], cs_c[0:64, tsl])
                    nc.vector.tensor_mul(
                        ks_t[0:64, :], pkv[0:64, :], cs_s[0:64, tsl])
                    kps = mmp.tile([128, S], F32, tag="mm", name=f"kd{strip}")
                    nc.tensor.matmul(kps, pk_sb[:, 0, :], kc_t[0:64, :],
                                     start=True, stop=False,
                                     skip_group_check=True)
                    nc.tensor.matmul(kps, pk_sb[:, 1, :], ks_t[0:64, :],
                                     start=False, stop=True,
                                     skip_group_check=True)
                    nc.vector.tensor_copy(kT[:, tsl], kps)
                    vt_s = rtmp.tile([128, S], BF16, tag="vt", name=f"vt{strip}")
                    nc.vector.tensor_copy(vt_s[64:128, :], pkv[64:128, :])
                    state["vt"] = vt_s

                for kc in range(KC):
                    def mk(kc=kc):
                        nc.tensor.matmul(
                            pkv, wkv_sb[kc // KG][:, kc % KG, :],
                            xa[strip, kc // KG][:, kc % KG, :],
                            start=(kc == 0), stop=(kc == KC - 1))
                        if kc == KC - 1:
                            rope_kv()
                    yield mk
                for n in range(4):
                    def mk(n=n):
                        if n == 0:
                            state["pt"] = tpp.tile([128, 4, 128], BF16,
                                                   tag="tp", name=f"pt{strip}")
                        pt = state["pt"]
                        nc.tensor.transpose(
                            pt[:, n, 0:64],
                            state["vt"][64:128, n * 128:(n + 1) * 128],
                            ident[64:128, 64:128])
                        nc.vector.tensor_copy(
                            vaug[:, strip * 4 + n, 0:64], pt[:, n, 0:64])
                    yield mk

            def oproj_filler(strip, ytn):
                """Yield closures, one o_proj matmul each; eviction + store
                ride along after each accumulation group."""
                t0 = strip * S
                for tsub in range(4):
                    trow = t0 + tsub * 128
                    stage = stgp.tile([128, 4, S], BF16, tag="st",
                                      name=f"st{strip}{tsub}")
                    for n in range(4):
                        po = mmp.tile([128, S], F32, tag="mm",
                                      name=f"po{strip}{tsub}{n}")
                        for c in range(2):
                            def mk(po=po, c=c, tsub=tsub, n=n, trow=trow,
                                   stage=stage):
                                nc.tensor.matmul(
                                    po,
                                    ytn[c][:, tsub * 128:(tsub + 1) * 128],
                                    wo_sb[:, c, n * S:(n + 1) * S],
                                    start=(c == 0), stop=(c == 1),
                                    skip_group_check=True)
                                if c == 1:
                                    # evictions on DVE: the ACT engine is the
                                    # binding engine during late-strip
                                    # attention (2 EXPs per j > PE per-j work)
                                    nc.vector.tensor_copy(
                                        stage[:, n, :], po)
                                    if n == 3:
                                        nc.gpsimd.dma_start(
                                            out=out[trow:trow + 128, :],
                                            in_=stage)
                            yield mk

            def run_filler(filler, frac):
                import itertools
                for fn in itertools.islice(filler, frac):
                    fn()

            # strip 0 projection runs dense (nothing to overlap with)
            for fn in proj_filler(0):
                fn()

            ytn_strips = {}
            fillers = []

            for strip in range(NSTRIP):
                t0 = strip * S
                n_sc = (strip + 1) * 4
                ytn = [ytnp.tile([128, S], BF16, tag="ytn",
                                 name=f"ytn{strip}{c}") for c in range(2)]
                ytn_strips[strip] = ytn

                pending = 0
                if strip + 1 < NSTRIP:
                    fillers.append(proj_filler(strip + 1))
                    pending += 52
                if strip - 1 >= 0:
                    fillers.append(oproj_filler(strip - 1, ytn_strips[strip - 1]))
                    pending += 32
                n_chunks = 2 * n_sc

                import itertools
                filler_iter = itertools.chain(*fillers)
                fillers = [filler_iter]
                per_chunk = -(-pending // n_chunks) if pending else 0

                for hp in range(2):
                    yt = ytp.tile([128, 8, 128], F32, tag="yt",
                                  name=f"yt{strip}{hp}")
                    # zero both banks of yt with one full-bank matmul each
                    for half in range(2):
                        nc.tensor.matmul(
                            yt[:, half * 4:(half + 1) * 4, :],
                            zeros_b, junk,
                            start=True, stop=True, skip_group_check=True)
                    def emit_av(j, o, ex):
                        for h in range(2):
                            for tb in range(o // 128, 4):
                                nc.tensor.matmul(
                                    yt[:, h * 4 + tb, 0:65],
                                    ex[h][:, tb * 128:(tb + 1) * 128],
                                    vaug[:, j, 0:65],
                                    start=False,
                                    stop=(j == 4 * strip + tb),
                                    skip_group_check=True)

                    prev = None
                    for j in range(n_sc):
                        o = max(j * 128 - t0, 0)
                        jsl = slice(j * 128, (j + 1) * 128)
                        ex = [expp.tile([128, S], BF16, tag="exp",
                                        name=f"e{strip}{hp}{j}{h}")
                              for h in range(2)]
                        sc = [mmp.tile([128, S], F32, tag="mm",
                                       name=f"s{strip}{hp}{j}{h}")
                              for h in range(2)]
                        # both head matmuls adjacent: they occupy disjoint
                        # 64-row groups of the PE and run concurrently
                        for h in range(2):
                            lo = h * 64
                            nc.tensor.matmul(
                                sc[h][:, o:S],
                                kT[lo:lo + 64, jsl],
                                qT[hp][lo:lo + 64, t0 + o:t0 + S],
                                start=True, stop=True)
                        for h in range(2):
                            nc.scalar.activation(
                                ex[h][:, o:S], sc[h][:, o:S],
                                mybir.ActivationFunctionType.Exp,
                                scale=1.0 / math.sqrt(HD))
                            if j * 128 - t0 >= 0:
                                nc.gpsimd.affine_select(
                                    out=ex[h][:, o:o + 128],
                                    in_=ex[h][:, o:o + 128],
                                    pattern=[[1, 128]], base=0,
                                    channel_multiplier=-1,
                                    compare_op=mybir.AluOpType.is_ge,
                                    fill=0.0)
                        # attn@v for the previous j: its exps are ready, so
                        # the PE never waits on the ACT engine
                        if prev is not None:
                            emit_av(*prev)
                        prev = (j, o, ex)
                        if debug and strip == 0 and hp == 0:
                            nc.sync.dma_start(
                                out=dbg["d_ex"][:, j * S:(j + 1) * S],
                                in_=ex[0])
                        run_filler(filler_iter, per_chunk)
                    emit_av(*prev)

                    # normalize by the denominator in column 64, pack the
                    # head pair side by side, transpose back to [d, t]
                    if debug and strip == 0 and hp == 0:
                        ydbg = rtmp.tile([128, 8 * 128], F32, tag="yd",
                                         bufs=1, name="ydbg")
                        nc.vector.tensor_copy(
                            ydbg, yt.rearrange("p a b -> p (a b)"))
                        nc.sync.dma_start(out=dbg["d_yt"], in_=ydbg)
                    last = (strip == NSTRIP - 1 and hp == 1)
                    if last:
                        tail_iter = oproj_filler(strip, ytn)
                    tp = tpp.tile([128, 4, 128], BF16, tag="tp",
                                  name=f"tp{strip}{hp}")
                    for tb in range(4):
                        ypk = ypkp.tile([128, 128], BF16, tag="yp",
                                        name=f"yp{strip}{hp}{tb}")
                        for h in range(2):
                            rcp = rcpp.tile([128, 1], F32, tag="rc",
                                            name=f"rc{strip}{hp}{tb}{h}")
                            nc.vector.reciprocal_approx_fast(
                                rcp, yt[:, h * 4 + tb, 64:65])
                            nc.vector.tensor_scalar_mul(
                                ypk[:, h * 64:(h + 1) * 64],
                                yt[:, h * 4 + tb, 0:64], rcp)
                        nc.tensor.transpose(tp[:, tb, :], ypk, ident)
                        nc.vector.tensor_copy(
                            ytn[hp][:, tb * 128:(tb + 1) * 128],
                            tp[:, tb, :])
                        if last:
                            # last strip: o_proj for this t-block can start
                            # as soon as both pairs' ytn slices exist
                            run_filler(tail_iter, 8)

                if debug and strip == 0:
                    nc.sync.dma_start(out=dbg["d_qT0"], in_=qT[0])
                    nc.sync.dma_start(out=dbg["d_qT1"], in_=qT[1])
                    nc.sync.dma_start(out=dbg["d_kT"], in_=kT)
                    nc.sync.dma_start(
                        out=dbg["d_vaug"],
                        in_=vaug.rearrange("p a b -> p (a b)"))
                    nc.sync.dma_start(out=dbg["d_ytn0"], in_=ytn[0])
                    nc.sync.dma_start(out=dbg["d_ytn1"], in_=ytn[1])

                for fn in filler_iter:
                    fn()
                fillers = []

            # (the last strip's o_proj was interleaved into its pair-1
            # normalize loop above)

    nc.compile()
    return nc


_NC_CACHE = {}


def _get_nc(debug=False):
    if debug not in _NC_CACHE:
        _NC_CACHE[debug] = _build_kernel(debug=debug)
    return _NC_CACHE[debug]


def _host_consts():
    # signed rope permutation matrices: the pair rotation is
    # rot = cos-part + P @ sin-part, with the [-sin | +sin] sign pattern of
    # the x1/x2 halves baked into P so the sin table stays all-positive
    def swap64(m):
        return (m & ~63) | ((m + 32) & 63)

    def sgn(m):
        return -1.0 if (m % 64) < 32 else 1.0

    pq = np.zeros((128, 128), dtype=np.float32)
    for m in range(128):
        pq[swap64(m), m] = sgn(m)
    pk = np.zeros((64, 2, 128), dtype=np.float32)
    for m in range(128):
        pk[m % 64, 0, m] = 1.0
        pk[(m % 64 + 32) % 64, 1, m] = sgn(m)
    ident = np.eye(128, dtype=np.float32)

    theta = 1.0 / ROPE_BASE ** (np.arange(0, HD, 2, dtype=np.float64) / HD)
    ang = np.arange(T, dtype=np.float64)[None, :] * theta[:, None]  # [32, T]
    cosT = np.ascontiguousarray(np.cos(ang).astype(np.float32))
    sinT = np.ascontiguousarray(np.sin(ang).astype(np.float32))
    return (pq.astype(BF), pk.astype(BF), ident.astype(BF), cosT, sinT)


def _prep_inputs(x, wq, wk, wv, wo):
    """Host-side shard + layout prep. Returns per-core input maps."""
    x = np.asarray(x, dtype=np.float32).reshape(T, D)
    wq = np.asarray(wq, dtype=np.float32)
    wk = np.asarray(wk, dtype=np.float32)
    wv = np.asarray(wv, dtype=np.float32)
    wo = np.asarray(wo, dtype=np.float32)

    # pack x to the SBUF tile layout: row s*128+p, col kc*S+t holds
    # xT[kc*128+p, s*S+t]  (see kernel load_xa; 16KB contiguous rows)
    xp = x.T.reshape(4, KG, 128, NSTRIP, S).transpose(3, 0, 2, 1, 4)
    xP = np.ascontiguousarray(
        xp.reshape(NSTRIP * 4 * 128, KG * S)).astype(BF)

    def pack_w(wT, n):  # wT: [D, n] -> [4*128, KG*n]
        wp = wT.reshape(4, KG, 128, n).transpose(0, 2, 1, 3)
        return np.ascontiguousarray(wp.reshape(4 * 128, KG * n)).astype(BF)

    # head-dim permutation for rope: [even pair comps | odd pair comps]
    perm = np.concatenate([np.arange(0, HD, 2), np.arange(1, HD, 2)])
    pqM, pkM, idM, cosT, sinT = _host_consts()

    in_maps = []
    for c in range(NCORES):
        wq_c = wq[c * QC:(c + 1) * QC].reshape(HPC, HD, D)[:, perm, :]
        wq_c = wq_c.reshape(QC, D)
        wk_c = wk[c * HD:(c + 1) * HD][perm, :]
        wv_c = wv[c * HD:(c + 1) * HD]
        wkv_c = np.concatenate([wk_c, wv_c], axis=0)          # [128, D]
        wo_c = wo[:, c * QC:(c + 1) * QC]                      # [D, QC]
        in_maps.append({
            "xP": xP,
            "wqP": pack_w(np.ascontiguousarray(wq_c.T), QC),
            "wkvP": pack_w(np.ascontiguousarray(wkv_c.T), 128),
            "woT": np.ascontiguousarray(wo_c.T).astype(BF),
            "cosT": cosT,
            "sinT": sinT,
            "pqM": pqM,
            "pkM": pkM,
            "idM": idM,
        })
    return in_maps


def kernel(x, wq, wk, wv, wo):
    from concourse.bass_utils import run_bass_kernel_spmd

    nc = _get_nc()
    in_maps = _prep_inputs(x, wq, wk, wv, wo)
    res = run_bass_kernel_spmd(nc, in_maps, core_ids=list(range(NCORES)))
    acc = np.zeros((T, D), dtype=np.float64)
    for c in range(NCORES):
        acc += res.results[c]["out"].astype(np.float64)
    return acc.astype(np.float32).reshape(1, T, D)


# revision 38
# speedup vs baseline: 1.0109x; 1.0041x over previous
"""Causal self-attention (GQA + RoPE) for TRN2, sharded over 8 NeuronCores.

Sharding: tensor-parallel over heads. Each core owns 4 query heads and 1 KV
head (H=32, HKV=8 -> group size 4). Column-parallel q/k/v projections,
row-parallel o_proj; the final all-reduce over the 8 partial [T, D] outputs
happens on the host after the gather.

v2 design notes (vs the f32r baseline):
  - All matmul operands are bf16: FWL weight loads, half the SBUF/DMA
    bandwidth, and much lower PE power (the f32r version spent ~60% of its
    runtime HAM-throttled to 1.2 GHz).
  - Scores for an even/odd head pair run as two concurrent 64-row
    tile_position matmuls (kT is duplicated on both partition halves).
  - attn@v is computed transposed: stationary = ex [keys, t-block], moving =
    v-augmented [keys, 64 v dims + ones col] -> yt [t, 65] with the softmax
    denominator landing in column 64.  Normalization is then a per-partition
    reciprocal_approx_fast + tensor_scalar_mul on the DVE (no ACT table
    thrash, no PE broadcast, no cross-partition shifts).
  - RoPE pair rotation and the kT partition-duplication are PE permutation
    matmuls (q_rot = I@qc + P@qs accumulated in PSUM), not SBUF->SBUF DMAs.
  - Inputs are loaded with 4-chunk batched DMA descriptors, all issued
    up-front on the sync engine; output partials are staged to bf16 and
    stored 2048-cols-at-a-time from the gpsimd engine.
"""

import math

import numpy as np
import ml_dtypes

import concourse.bass as bass
import concourse.mybir as mybir
import concourse.tile as tile
from concourse import bacc

D = 2048
H = 32
HKV = 8
HD = 64
T = 2048
NCORES = 8
HPC = H // NCORES        # 4 query heads per core
QC = HPC * HD            # 256 q dims per core
ROPE_BASE = 10000.0
S = 512                  # t-strip width
NSTRIP = T // S          # 4
KC = D // 128            # 16 contraction chunks
KG = 4                   # kc chunks per batched DMA / SBUF tile

F32 = mybir.dt.float32
BF16 = mybir.dt.bfloat16

BF = ml_dtypes.bfloat16


def _build_kernel(debug=False):
    nc = bacc.Bacc("TRN2", target_bir_lowering=False, debug=False,
                   num_devices=NCORES)

    # xP/wqP/wkvP are host-packed to the exact SBUF tile layout so each DMA
    # descriptor row is one contiguous 2-4KB segment (the DMA engines are
    # descriptor-rate limited at ~74ns/row-segment, so segment size sets
    # the effective input bandwidth)
    xP = nc.dram_tensor("xP", [NSTRIP * 4 * 128, KG * S], BF16,
                        kind="ExternalInput").ap()
    wqP = nc.dram_tensor("wqP", [4 * 128, KG * QC], BF16,
                         kind="ExternalInput").ap()
    wkvP = nc.dram_tensor("wkvP", [4 * 128, KG * 128], BF16,
                          kind="ExternalInput").ap()
    woT = nc.dram_tensor("woT", [QC, D], BF16, kind="ExternalInput").ap()
    # compact rope tables: [32, T]; duplicated to 128 partitions on-chip
    # (the rotation sign pattern is baked into the permutation matrices)
    cosT = nc.dram_tensor("cosT", [32, T], F32, kind="ExternalInput").ap()
    sinT = nc.dram_tensor("sinT", [32, T], F32, kind="ExternalInput").ap()
    pqM = nc.dram_tensor("pqM", [128, 128], BF16, kind="ExternalInput").ap()
    pkM = nc.dram_tensor("pkM", [64, 2, 128], BF16, kind="ExternalInput").ap()
    idM = nc.dram_tensor("idM", [128, 128], BF16, kind="ExternalInput").ap()
    out = nc.dram_tensor("out", [T, D], BF16, kind="ExternalOutput").ap()
    dbg = {}
    if debug:
        for nm, shp, dt in [("d_qT0", [128, T], BF16), ("d_qT1", [128, T], BF16),
                            ("d_kT", [128, T], BF16),
                            ("d_vaug", [128, 16 * 72], BF16),
                            ("d_ex", [128, 4 * S], BF16),
                            ("d_yt", [128, 8 * 128], F32),
                            ("d_ytn0", [128, S], BF16),
                            ("d_ytn1", [128, S], BF16)]:
            dbg[nm] = nc.dram_tensor(nm, shp, dt, kind="ExternalOutput").ap()

    with tile.TileContext(nc) as tc:
        with (
            tc.tile_pool(name="consts", bufs=1) as consts,
            tc.tile_pool(name="persist", bufs=1) as persist,
            tc.tile_pool(name="rtmp", bufs=6) as rtmp,
            tc.tile_pool(name="expp", bufs=6) as expp,
            tc.tile_pool(name="ypk", bufs=3) as ypkp,
            tc.tile_pool(name="rcp", bufs=6) as rcpp,
            tc.tile_pool(name="ytn", bufs=4) as ytnp,
            tc.tile_pool(name="stg", bufs=3) as stgp,
            tc.tile_pool(name="mm", bufs=3, space="PSUM") as mmp,
            tc.tile_pool(name="ytp", bufs=2, space="PSUM") as ytp,
            tc.tile_pool(name="tpp", bufs=1, space="PSUM") as tpp,
        ):
            # ---- input DMAs: all issued up-front on the sync engine, in
            # criticality order (strip-0 weights+x first, wo last) ----
            ident = consts.tile([128, 128], BF16)
            nc.sync.dma_start(out=ident, in_=idM)
            pq_sb = consts.tile([128, 128], BF16)
            pk_sb = consts.tile([64, 2, 128], BF16)
            nc.sync.dma_start(out=pq_sb, in_=pqM)
            nc.sync.dma_start(out=pk_sb, in_=pkM)
            # rope tables first: everything downstream of rope blocks on them
            # (4 parallel block loads per table, no serial dup chain)
            cs_c = consts.tile([128, T], F32)
            cs_s = consts.tile([128, T], F32)
            for b in range(4):
                nc.sync.dma_start(out=cs_c[b * 32:(b + 1) * 32, :], in_=cosT)
                nc.sync.dma_start(out=cs_s[b * 32:(b + 1) * 32, :], in_=sinT)
            # per-group tiles (4 chunks each) with 2-4KB contiguous rows:
            # fine-grained enough that projections start as groups land,
            # coarse enough to stay byte-limited on the DMA pool
            wq_sb = [consts.tile([128, KG, QC], BF16, name=f"wq{g}")
                     for g in range(KC // KG)]
            wkv_sb = [consts.tile([128, KG, 128], BF16, name=f"wkv{g}")
                      for g in range(KC // KG)]
            xa = {}  # (strip, g) -> [128, KG, S] tile

            def load_xa(strip):
                for g in range(KC // KG):
                    xt = consts.tile([128, KG, S], BF16, name=f"xa{strip}_{g}")
                    r0 = (strip * 4 + g) * 128
                    nc.sync.dma_start(
                        out=xt.rearrange("p c t -> p (c t)"),
                        in_=xP[r0:r0 + 128, :])
                    xa[strip, g] = xt

            for g in range(KC // KG):
                nc.sync.dma_start(
                    out=wq_sb[g].rearrange("p c n -> p (c n)"),
                    in_=wqP[g * 128:(g + 1) * 128, :])
                nc.sync.dma_start(
                    out=wkv_sb[g].rearrange("p c n -> p (c n)"),
                    in_=wkvP[g * 128:(g + 1) * 128, :])
                xt = consts.tile([128, KG, S], BF16, name=f"xa0_{g}")
                nc.sync.dma_start(
                    out=xt.rearrange("p c t -> p (c t)"),
                    in_=xP[g * 128:(g + 1) * 128, :])
                xa[0, g] = xt
            load_xa(1)
            load_xa(2)
            wo_sb = consts.tile([128, 2, D], BF16)
            nc.sync.dma_start(
                out=wo_sb, in_=woT.rearrange("(c p) n -> p c n", p=128))
            load_xa(3)

            # PE warmup: lift the HAM cold throttle while input DMAs land
            junk = consts.tile([128, 512], BF16)
            nc.vector.memset(junk, 1.0)
            # zeros stationary: used to clear whole PSUM banks in one matmul
            # (start=True marks the full 2KB bank pending-zero, so packing
            # several accumulation slots per bank requires a single
            # bank-covering start op; the per-slot matmuls then accumulate
            # with start=False)
            zeros_b = consts.tile([128, 128], BF16)
            nc.vector.memset(zeros_b, 0.0)
            # enough warmup to both lift the HAM throttle and keep the PE
            # busy while the first ~7MB of inputs stream in (~25us at the
            # observed ~270GB/s aggregate DMA rate)
            warm_ps = mmp.tile([128, 512], F32, tag="mm", name="warm")
            for w in range(44):
                nc.tensor.matmul(
                    warm_ps, junk[:, 0:128], junk,
                    start=True, stop=True, skip_group_check=True)

            # persistent activations
            qT = [persist.tile([128, T], BF16, name=f"qT{i}") for i in range(2)]
            # k duplicated on both partition halves so the head-pair scores
            # run as two concurrent 64-row tile_position matmuls
            kT = persist.tile([128, T], BF16)
            vaug = persist.tile([128, 4 * NSTRIP, 72], BF16)
            ones_f = consts.tile([128, 4 * NSTRIP, 1], F32)
            nc.vector.memset(ones_f, 1.0)
            nc.vector.tensor_copy(vaug[:, :, 64:65], ones_f)

            def proj_filler(strip):
                """Yield closures, one PE op each, for this strip's q/kv
                projection; rope + v-transpose work rides along."""
                t0 = strip * S
                tsl = slice(t0, t0 + S)

                def rope_q(hp, pq):
                    qc = rtmp.tile([128, S], F32, tag="rtf", bufs=3,
                                   name=f"qc{strip}{hp}")
                    qs = rtmp.tile([128, S], BF16, tag="rt", name=f"qs{strip}{hp}")
                    nc.vector.tensor_mul(qc, pq, cs_c[:, tsl])
                    nc.vector.tensor_mul(qs, pq, cs_s[:, tsl])
                    rps = mmp.tile([128, S], F32, tag="mm", name=f"rq{strip}{hp}")
                    nc.tensor.matmul(rps, pq_sb, qs, start=True, stop=True,
                                     skip_group_check=True)
                    nc.vector.tensor_add(qT[hp][:, tsl], qc, rps)

                for hp in range(2):
                    pq = mmp.tile([128, S], F32, tag="mm", name=f"pq{strip}_{hp}")
                    for kc in range(KC):
                        def mk(hp=hp, pq=pq, kc=kc):
                            nc.tensor.matmul(
                                pq,
                                wq_sb[kc // KG][:, kc % KG,
                                                hp * 128:(hp + 1) * 128],
                                xa[strip, kc // KG][:, kc % KG, :],
                                start=(kc == 0), stop=(kc == KC - 1))
                            if kc == KC - 1:
                                rope_q(hp, pq)
                        yield mk

                pkv = mmp.tile([128, S], F32, tag="mm", name=f"pkv{strip}")
                state = {}

                def rope_kv():
                    kc_t = rtmp.tile([128, S], BF16, tag="rt", name=f"kc{strip}")
                    ks_t = rtmp.tile([128, S], BF16, tag="rt", name=f"ks{strip}")
                    nc.vector.tensor_mul(
                        kc_t[0:64, :], pkv[0:64, :], cs_c[0:64, tsl])
                    nc.vector.tensor_mul(
                        ks_t[0:64, :], pkv[0:64, :], cs_s[0:64, tsl])
                    kps = mmp.tile([128, S], F32, tag="mm", name=f"kd{strip}")
                    nc.tensor.matmul(kps, pk_sb[:, 0, :], kc_t[0:64, :],
                                     start=True, stop=False,
                                     skip_group_check=True)
                    nc.tensor.matmul(kps, pk_sb[:, 1, :], ks_t[0:64, :],
                                     start=False, stop=True,
                                     skip_group_check=True)
                    nc.vector.tensor_copy(kT[:, tsl], kps)
                    vt_s = rtmp.tile([128, S], BF16, tag="vt", name=f"vt{strip}")
                    nc.vector.tensor_copy(vt_s[64:128, :], pkv[64:128, :])
                    state["vt"] = vt_s

                for kc in range(KC):
                    def mk(kc=kc):
                        nc.tensor.matmul(
                            pkv, wkv_sb[kc // KG][:, kc % KG, :],
                            xa[strip, kc // KG][:, kc % KG, :],
                            start=(kc == 0), stop=(kc == KC - 1))
                        if kc == KC - 1:
                            rope_kv()
                    yield mk
                for n in range(4):
                    def mk(n=n):
                        if n == 0:
                            state["pt"] = tpp.tile([128, 4, 128], BF16,
                                                   tag="tp", name=f"pt{strip}")
                        pt = state["pt"]
                        nc.tensor.transpose(
                            pt[:, n, 0:64],
                            state["vt"][64:128, n * 128:(n + 1) * 128],
                            ident[64:128, 64:128])
                        nc.vector.tensor_copy(
                            vaug[:, strip * 4 + n, 0:64], pt[:, n, 0:64])
                    yield mk

            def oproj_filler(strip, ytn):
                """Yield closures, one o_proj matmul each; eviction + store
                ride along after each accumulation group."""
                t0 = strip * S
                for tsub in range(4):
                    trow = t0 + tsub * 128
                    stage = stgp.tile([128, 4, S], BF16, tag="st",
                                      name=f"st{strip}{tsub}")
                    for n in range(4):
                        po = mmp.tile([128, S], F32, tag="mm",
                                      name=f"po{strip}{tsub}{n}")
                        for c in range(2):
                            def mk(po=po, c=c, tsub=tsub, n=n, trow=trow,
                                   stage=stage):
                                nc.tensor.matmul(
                                    po,
                                    ytn[c][:, tsub * 128:(tsub + 1) * 128],
                                    wo_sb[:, c, n * S:(n + 1) * S],
                                    start=(c == 0), stop=(c == 1),
                                    skip_group_check=True)
                                if c == 1:
                                    # evictions on DVE: the ACT engine is the
                                    # binding engine during late-strip
                                    # attention (2 EXPs per j > PE per-j work)
                                    nc.vector.tensor_copy(
                                        stage[:, n, :], po)
                                    if n == 3:
                                        nc.gpsimd.dma_start(
                                            out=out[trow:trow + 128, :],
                                            in_=stage)
                            yield mk

            def run_filler(filler, frac):
                import itertools
                for fn in itertools.islice(filler, frac):
                    fn()

            # strip 0 projection runs dense (nothing to overlap with)
            for fn in proj_filler(0):
                fn()

            ytn_strips = {}
            fillers = []

            for strip in range(NSTRIP):
                t0 = strip * S
                n_sc = (strip + 1) * 4
                ytn = [ytnp.tile([128, S], BF16, tag="ytn",
                                 name=f"ytn{strip}{c}") for c in range(2)]
                ytn_strips[strip] = ytn

                pending = 0
                if strip + 1 < NSTRIP:
                    fillers.append(proj_filler(strip + 1))
                    pending += 52
                if strip - 1 >= 0:
                    fillers.append(oproj_filler(strip - 1, ytn_strips[strip - 1]))
                    pending += 32
                n_chunks = 2 * n_sc

                import itertools
                filler_iter = itertools.chain(*fillers)
                fillers = [filler_iter]
                per_chunk = -(-pending // n_chunks) if pending else 0

                for hp in range(2):
                    yt = ytp.tile([128, 8, 128], F32, tag="yt",
                                  name=f"yt{strip}{hp}")
                    # zero both banks of yt with one full-bank matmul each
                    for half in range(2):
                        nc.tensor.matmul(
                            yt[:, half * 4:(half + 1) * 4, :],
                            zeros_b, junk,
                            start=True, stop=True, skip_group_check=True)
                    def emit_av(j, o, ex):
                        for h in range(2):
                            for tb in range(o // 128, 4):
                                nc.tensor.matmul(
                                    yt[:, h * 4 + tb, 0:65],
                                    ex[h][:, tb * 128:(tb + 1) * 128],
                                    vaug[:, j, 0:65],
                                    start=False,
                                    stop=(j == 4 * strip + tb),
                                    skip_group_check=True)

                    prev = None
                    for j in range(n_sc):
                        o = max(j * 128 - t0, 0)
                        jsl = slice(j * 128, (j + 1) * 128)
                        ex = [expp.tile([128, S], BF16, tag="exp",
                                        name=f"e{strip}{hp}{j}{h}")
                              for h in range(2)]
                        sc = [mmp.tile([128, S], F32, tag="mm",
                                       name=f"s{strip}{hp}{j}{h}")
                              for h in range(2)]
                        # both head matmuls adjacent: they occupy disjoint
                        # 64-row groups of the PE and run concurrently
                        for h in range(2):
                            lo = h * 64
                            nc.tensor.matmul(
                                sc[h][:, o:S],
                                kT[lo:lo + 64, jsl],
                                qT[hp][lo:lo + 64, t0 + o:t0 + S],
                                start=True, stop=True)
                        for h in range(2):
                            nc.scalar.activation(
                                ex[h][:, o:S], sc[h][:, o:S],
                                mybir.ActivationFunctionType.Exp,
                                scale=1.0 / math.sqrt(HD))
                            if j * 128 - t0 >= 0:
                                nc.gpsimd.affine_select(
                                    out=ex[h][:, o:o + 128],
                                    in_=ex[h][:, o:o + 128],
                                    pattern=[[1, 128]], base=0,
                                    channel_multiplier=-1,
                                    compare_op=mybir.AluOpType.is_ge,
                                    fill=0.0)
                        # attn@v for the previous j: its exps are ready, so
                        # the PE never waits on the ACT engine
                        if prev is not None:
                            emit_av(*prev)
                        prev = (j, o, ex)
                        if debug and strip == 0 and hp == 0:
                            nc.sync.dma_start(
                                out=dbg["d_ex"][:, j * S:(j + 1) * S],
                                in_=ex[0])
                        run_filler(filler_iter, per_chunk)
                    emit_av(*prev)

                    # normalize by the denominator in column 64, pack the
                    # head pair side by side, transpose back to [d, t]
                    if debug and strip == 0 and hp == 0:
                        ydbg = rtmp.tile([128, 8 * 128], F32, tag="yd",
                                         bufs=1, name="ydbg")
                        nc.vector.tensor_copy(
                            ydbg, yt.rearrange("p a b -> p (a b)"))
                        nc.sync.dma_start(out=dbg["d_yt"], in_=ydbg)
                    last = (strip == NSTRIP - 1 and hp == 1)
                    if last:
                        tail_iter = oproj_filler(strip, ytn)
                    tp = tpp.tile([128, 4, 128], BF16, tag="tp",
                                  name=f"tp{strip}{hp}")
                    for tb in range(4):
                        ypk = ypkp.tile([128, 128], BF16, tag="yp",
                                        name=f"yp{strip}{hp}{tb}")
                        for h in range(2):
                            rcp = rcpp.tile([128, 1], F32, tag="rc",
                                            name=f"rc{strip}{hp}{tb}{h}")
                            nc.vector.reciprocal_approx_fast(
                                rcp, yt[:, h * 4 + tb, 64:65])
                            nc.vector.tensor_scalar_mul(
                                ypk[:, h * 64:(h + 1) * 64],
                                yt[:, h * 4 + tb, 0:64], rcp)
                        nc.tensor.transpose(tp[:, tb, :], ypk, ident)
                        nc.vector.tensor_copy(
                            ytn[hp][:, tb * 128:(tb + 1) * 128],
                            tp[:, tb, :])
                        if last:
                            # last strip: o_proj for this t-block can start
                            # as soon as both pairs' ytn slices exist
                            run_filler(tail_iter, 8)

                if debug and strip == 0:
                    nc.sync.dma_start(out=dbg["d_qT0"], in_=qT[0])
                    nc.sync.dma_start(out=dbg["d_qT1"], in_=qT[1])
                    nc.sync.dma_start(out=dbg["d_kT"], in_=kT)
                    nc.sync.dma_start(
                        out=dbg["d_vaug"],
                        in_=vaug.rearrange("p a b -> p (a b)"))
                    nc.sync.dma_start(out=dbg["d_ytn0"], in_=ytn[0])
                    nc.sync.dma_start(out=dbg["d_ytn1"], in_=ytn[1])

                for fn in filler_iter:
                    fn()
                fillers = []

            # (the last strip's o_proj was interleaved into its pair-1
            # normalize loop above)

    nc.compile()
    return nc


_NC_CACHE = {}


def _get_nc(debug=False):
    if debug not in _NC_CACHE:
        _NC_CACHE[debug] = _build_kernel(debug=debug)
    return _NC_CACHE[debug]


def _host_consts():
    # signed rope permutation matrices: the pair rotation is
    # rot = cos-part + P @ sin-part, with the [-sin | +sin] sign pattern of
    # the x1/x2 halves baked into P so the sin table stays all-positive
    def swap64(m):
        return (m & ~63) | ((m + 32) & 63)

    def sgn(m):
        return -1.0 if (m % 64) < 32 else 1.0

    pq = np.zeros((128, 128), dtype=np.float32)
    for m in range(128):
        pq[swap64(m), m] = sgn(m)
    pk = np.zeros((64, 2, 128), dtype=np.float32)
    for m in range(128):
        pk[m % 64, 0, m] = 1.0
        pk[(m % 64 + 32) % 64, 1, m] = sgn(m)
    ident = np.eye(128, dtype=np.float32)

    theta = 1.0 / ROPE_BASE ** (np.arange(0, HD, 2, dtype=np.float64) / HD)
    ang = np.arange(T, dtype=np.float64)[None, :] * theta[:, None]  # [32, T]
    cosT = np.ascontiguousarray(np.cos(ang).astype(np.float32))
    sinT = np.ascontiguousarray(np.sin(ang).astype(np.float32))
    return (pq.astype(BF), pk.astype(BF), ident.astype(BF), cosT, sinT)


def _prep_inputs(x, wq, wk, wv, wo):
    """Host-side shard + layout prep. Returns per-core input maps."""
    x = np.asarray(x, dtype=np.float32).reshape(T, D)
    wq = np.asarray(wq, dtype=np.float32)
    wk = np.asarray(wk, dtype=np.float32)
    wv = np.asarray(wv, dtype=np.float32)
    wo = np.asarray(wo, dtype=np.float32)

    # pack x to the SBUF tile layout: row s*128+p, col kc*S+t holds
    # xT[kc*128+p, s*S+t]  (see kernel load_xa; 16KB contiguous rows)
    xp = x.T.reshape(4, KG, 128, NSTRIP, S).transpose(3, 0, 2, 1, 4)
    xP = np.ascontiguousarray(
        xp.reshape(NSTRIP * 4 * 128, KG * S)).astype(BF)

    def pack_w(wT, n):  # wT: [D, n] -> [4*128, KG*n]
        wp = wT.reshape(4, KG, 128, n).transpose(0, 2, 1, 3)
        return np.ascontiguousarray(wp.reshape(4 * 128, KG * n)).astype(BF)

    # head-dim permutation for rope: [even pair comps | odd pair comps]
    perm = np.concatenate([np.arange(0, HD, 2), np.arange(1, HD, 2)])
    pqM, pkM, idM, cosT, sinT = _host_consts()

    in_maps = []
    for c in range(NCORES):
        wq_c = wq[c * QC:(c + 1) * QC].reshape(HPC, HD, D)[:, perm, :]
        wq_c = wq_c.reshape(QC, D)
        wk_c = wk[c * HD:(c + 1) * HD][perm, :]
        wv_c = wv[c * HD:(c + 1) * HD]
        wkv_c = np.concatenate([wk_c, wv_c], axis=0)          # [128, D]
        wo_c = wo[:, c * QC:(c + 1) * QC]                      # [D, QC]
        in_maps.append({
            "xP": xP,
            "wqP": pack_w(np.ascontiguousarray(wq_c.T), QC),
            "wkvP": pack_w(np.ascontiguousarray(wkv_c.T), 128),
            "woT": np.ascontiguousarray(wo_c.T).astype(BF),
            "cosT": cosT,
            "sinT": sinT,
            "pqM": pqM,
            "pkM": pkM,
            "idM": idM,
        })
    return in_maps


def kernel(x, wq, wk, wv, wo):
    from concourse.bass_utils import run_bass_kernel_spmd

    nc = _get_nc()
    in_maps = _prep_inputs(x, wq, wk, wv, wo)
    res = run_bass_kernel_spmd(nc, in_maps, core_ids=list(range(NCORES)))
    acc = np.zeros((T, D), dtype=np.float64)
    for c in range(NCORES):
        acc += res.results[c]["out"].astype(np.float64)
    return acc.astype(np.float32).reshape(1, T, D)
